# revision 1
# baseline (speedup 1.0000x reference)
"""Trainium2 8-core attention kernel.

Problem: B=2, H=16, S=2048, D=64 dense attention, f32 I/O.
Sharding: B*H = 32 head-batches -> 4 heads per NeuronCore (embarrassingly
parallel, no collectives).

Per-core algorithm (all in transposed score space, zero on-chip transposes
until the tiny epilogue):
  S^T[k, q] = (K d-major) . (Q d-major)   via PE matmul, contraction d=64
  P = exp(S^T / 8)                         via ScalarE (scale folded into ACT)
  outT[d', q] = V'^T @ P                   via PE matmul, contraction k
      where V' = [V | ones] so outT row 64 = softmax denominator
  out[q, d] = outT[:64].T / den            PE transpose + DVE recip/mul

Host side only reshapes/transposes/casts (layout choices for sharding):
  qt, kt: [4, 128, 2048] bf16  (d on partitions, zero-padded 64->128)
  vp:     [4, 128, 16, 65] bf16 (k%128 on partitions, ones column appended)
  out:    [4, 2048, 64] f32 natural layout
"""

import numpy as np
import ml_dtypes

import concourse.bass as bass
import concourse.tile as tile
from concourse import bacc, mybir
from concourse.bass_utils import run_bass_kernel_spmd
from concourse.masks import make_identity

B, H, S, D = 2, 16, 2048, 64
NCORES = 8
HPC = (B * H) // NCORES  # heads per core = 4
P = 128
KT = S // P  # 16 k-tiles
QC = S // 512  # 4 q-chunks of 512
SCALE = 1.0 / np.sqrt(D)  # 0.125

f32 = mybir.dt.float32
bf16 = mybir.dt.bfloat16


def build_head(tc, pools, aps, h):
    """Issue QK^T + exp for head h."""
    nc = tc.nc
    qt, kt, vp, out = aps
    qk_pool, v_pool, p_pool, epi_pool, ps_s, ps_o, ident = pools

    qt_b = qk_pool.tile([P, S], bf16, tag="qt")
    nc.sync.dma_start(qt_b[:], qt[h])
    kt_b = qk_pool.tile([P, S], bf16, tag="kt")
    nc.sync.dma_start(kt_b[:], kt[h])
    v_b = v_pool.tile([P, KT, D + 1], bf16, tag="v")
    nc.sync.dma_start(v_b[:], vp[h])
    p_b = p_pool.tile([P, KT, S], bf16, tag="p")

    # scores (transposed layout) + exp
    for kt_i in range(KT):
        for half in range(2):
            s_ps = ps_s.tile([P, 1024], f32, tag="s")
            for j in range(2):
                qlo = half * 1024 + j * 512
                nc.tensor.matmul(
                    s_ps[:, j * 512 : (j + 1) * 512],
                    lhsT=kt_b[:, kt_i * P : (kt_i + 1) * P],
                    rhs=qt_b[:, qlo : qlo + 512],
                    start=True,
                    stop=True,
                )
            nc.scalar.activation(
                p_b[:, kt_i, half * 1024 : (half + 1) * 1024],
                s_ps[:],
                mybir.ActivationFunctionType.Exp,
                scale=float(SCALE),
            )
    return p_b, v_b


def build_pv(tc, pools, aps, h, p_b, v_b):
    """Issue PV + epilogue for head h."""
    nc = tc.nc
    qt, kt, vp, out = aps
    qk_pool, v_pool, p_pool, epi_pool, ps_s, ps_o, ident = pools

    for qc in range(QC):
        o_ps = ps_o.tile([P, 512], f32, tag="o")
        for kt_i in range(KT):
            nc.tensor.matmul(
                o_ps[: D + 1, :],
                lhsT=v_b[:, kt_i, :],
                rhs=p_b[:, kt_i, qc * 512 : (qc + 1) * 512],
                start=(kt_i == 0),
                stop=(kt_i == KT - 1),
            )
        ot_sb = epi_pool.tile([D + 1, 512], f32, tag="ot")
        nc.vector.tensor_copy(ot_sb[:], o_ps[: D + 1, :])
        for c in range(4):
            tr_ps = ps_o.tile([P, D + 1], f32, tag="tr")
            nc.tensor.transpose(
                tr_ps[:], ot_sb[:, c * P : (c + 1) * P], ident[: D + 1, : D + 1]
            )
            rden = epi_pool.tile([P, 1], f32, tag="rden")
            nc.vector.reciprocal_approx_fast(rden[:], tr_ps[:, D : D + 1])
            o_sb = epi_pool.tile([P, D], f32, tag="osb")
            nc.vector.tensor_mul(
                o_sb[:], tr_ps[:, :D], rden[:, 0:1].to_broadcast((P, D))
            )
            q0 = qc * 512 + c * P
            nc.sync.dma_start(out[h, q0 : q0 + P, :], o_sb[:])


def build_nc():
    nc = bacc.Bacc("TRN2", target_bir_lowering=False, debug=False)
    qt = nc.dram_tensor("qt", [HPC, P, S], bf16, kind="ExternalInput").ap()
    kt = nc.dram_tensor("kt", [HPC, P, S], bf16, kind="ExternalInput").ap()
    vp = nc.dram_tensor("vp", [HPC, P, KT, D + 1], bf16, kind="ExternalInput").ap()
    out = nc.dram_tensor("out", [HPC, S, D], f32, kind="ExternalOutput").ap()
    aps = (qt, kt, vp, out)

    with tile.TileContext(nc) as tc:
        with (
            tc.tile_pool(name="const", bufs=1) as const_pool,
            tc.tile_pool(name="qk", bufs=2) as qk_pool,
            tc.tile_pool(name="v", bufs=2) as v_pool,
            tc.tile_pool(name="p", bufs=2) as p_pool,
            tc.tile_pool(name="epi", bufs=3) as epi_pool,
            tc.tile_pool(name="ps_s", bufs=2, space="PSUM") as ps_s,
            tc.tile_pool(name="ps_o", bufs=2, space="PSUM") as ps_o,
        ):
            ident = const_pool.tile([P, P], f32)
            make_identity(nc, ident[:])
            pools = (qk_pool, v_pool, p_pool, epi_pool, ps_s, ps_o, ident)

            # Interleave heads: QK+exp of head h is issued before PV of head
            # h-1 so the PE never sits behind the ScalarE exp stream.
            prev = None
            for h in range(HPC):
                cur = build_head(tc, pools, aps, h)
                if prev is not None:
                    build_pv(tc, pools, aps, h - 1, *prev)
                prev = cur
            build_pv(tc, pools, aps, HPC - 1, *prev)

    nc.compile()
    return nc


def shard_inputs(Q, K, V):
    """Full [B,H,S,D] f32 -> per-core input maps (layout + dtype choices)."""
    Qh = np.asarray(Q, dtype=np.float32).reshape(B * H, S, D)
    Kh = np.asarray(K, dtype=np.float32).reshape(B * H, S, D)
    Vh = np.asarray(V, dtype=np.float32).reshape(B * H, S, D)

    in_maps = []
    for c in range(NCORES):
        sl = slice(c * HPC, (c + 1) * HPC)
        qt = np.zeros((HPC, P, S), dtype=ml_dtypes.bfloat16)
        kt = np.zeros((HPC, P, S), dtype=ml_dtypes.bfloat16)
        qt[:, :D, :] = Qh[sl].transpose(0, 2, 1).astype(ml_dtypes.bfloat16)
        kt[:, :D, :] = Kh[sl].transpose(0, 2, 1).astype(ml_dtypes.bfloat16)
        vp = np.ones((HPC, S, D + 1), dtype=np.float32)
        vp[:, :, :D] = Vh[sl]
        # [h, (kt p), d] -> [h, p, kt, d']
        vp = (
            vp.reshape(HPC, KT, P, D + 1)
            .transpose(0, 2, 1, 3)
            .astype(ml_dtypes.bfloat16)
        )
        in_maps.append({"qt": np.ascontiguousarray(qt),
                        "kt": np.ascontiguousarray(kt),
                        "vp": np.ascontiguousarray(vp)})
    return in_maps


_NC_CACHE = None


def kernel(Q, K, V):
    global _NC_CACHE
    if _NC_CACHE is None:
        _NC_CACHE = build_nc()
    nc = _NC_CACHE
    in_maps = shard_inputs(Q, K, V)
    res = run_bass_kernel_spmd(nc, in_maps, core_ids=list(range(NCORES)))
    out = np.empty((B * H, S, D), dtype=np.float32)
    for c in range(NCORES):
        out[c * HPC : (c + 1) * HPC] = res.results[c]["out"]
    return out.reshape(B, H, S, D)


if __name__ == "__main__":
    nc = build_nc()
    print("compiled OK")


# revision 2
# speedup vs baseline: 1.1032x; 1.1032x over previous
"""Trainium2 8-core attention kernel (v2).

Problem: B=2, H=16, S=2048, D=64 dense attention, f32 I/O.
Sharding: B*H = 32 head-batches -> 4 heads per NeuronCore (embarrassingly
parallel, no collectives).

Per-core algorithm (transposed score space, no transposes until epilogue):
  S^T[k, q] = K_dmaj . Q_dmaj      PE matmul, contraction d=64, ROW-TILED:
                                   two concurrent matmuls in row-groups
                                   (0,0)/(64,0) with Q,K duplicated on
                                   partitions 64-127 (2x QK throughput)
  P = exp(S^T / 8)                 hybrid: ScalarE ACT exp for kt 0..10,
                                   VectorE Schraudolph bits (one tensor_scalar
                                   f32->int16 round(A*s+B), bitcast to bf16)
                                   for kt 11..15 -- splits the exp bottleneck
                                   across two engines
  outT[d', q] = V'^T @ P           PE matmul, contraction k; V'=[V|ones] so
                                   row 64 = softmax denominator
  out[q, d] = outT[:64].T / den    PE transpose + DVE recip_approx + mul

Host side only reshapes/transposes/casts (layout choices for sharding):
  qt, kt: [4, 128, 2048] bf16 (d on partitions, rows 64:128 duplicate 0:64)
  vp:     [4, 128, 16, 65] bf16 (k%128 on partitions, ones column appended)
  out:    [4, 2048, 64] f32 natural layout
"""

import numpy as np
import ml_dtypes

import concourse.bass as bass
import concourse.tile as tile
from concourse import bacc, mybir
from concourse.bass_utils import run_bass_kernel_spmd
from concourse.masks import make_identity

B, H, S, D = 2, 16, 2048, 64
NCORES = 8
HPC = (B * H) // NCORES  # heads per core = 4
P = 128
KT = S // P  # 16 k-tiles
QC = S // 512  # 4 q-chunks of 512
SCALE = 1.0 / np.sqrt(D)  # 0.125

# Schraudolph bf16-exp constants: bits16 = round(A*s + B); bitcast -> bf16
SCH_A = float(P * np.log2(np.e) * SCALE)
SCH_B = float(P * 127 - 7.5)
ACT_KT = list(range(11))          # kt tiles exp'd on ScalarE
DVE_KT = list(range(11, KT))      # kt tiles exp'd on VectorE (Schraudolph)

f32 = mybir.dt.float32
bf16 = mybir.dt.bfloat16
i16 = mybir.dt.int16


def emit_loads(nc, pools, aps, h):
    qt, kt, vp, out = aps
    qk_pool, v_pool, p_pool, epi_pool, ps_s, ps_o, ps_tr, ident = pools
    qt_b = qk_pool.tile([P, S], bf16, tag="qt")
    kt_b = qk_pool.tile([P, S], bf16, tag="kt")
    # split loads so the first QK tile's deps land early
    nc.sync.dma_start(kt_b[:, : S // 2], kt[h, :, : S // 2])
    nc.sync.dma_start(qt_b[:, : S // 2], qt[h, :, : S // 2])
    nc.sync.dma_start(kt_b[:, S // 2 :], kt[h, :, S // 2 :])
    nc.sync.dma_start(qt_b[:, S // 2 :], qt[h, :, S // 2 :])
    v_b = v_pool.tile([P, KT, D + 1], bf16, tag="v")
    nc.sync.dma_start(v_b[:], vp[h])
    p_b = p_pool.tile([P, KT, S], bf16, tag="p")
    return qt_b, kt_b, v_b, p_b


def emit_scores_half(nc, pools, h, half, qt_b, kt_b, p_b):
    """QK^T (row-tiled pairs) + exp for q-range [half*1024, (half+1)*1024)."""
    qk_pool, v_pool, p_pool, epi_pool, ps_s, ps_o, ps_tr, ident = pools
    q0 = half * 1024
    for kt_i in range(KT):
        s_ps = ps_s.tile([P, 1024], f32, tag="s")
        nc.tensor.matmul(
            s_ps[:, 0:512],
            lhsT=kt_b[0:64, kt_i * P : (kt_i + 1) * P],
            rhs=qt_b[0:64, q0 : q0 + 512],
            start=True,
            stop=True,
            tile_position=(0, 0),
        )
        nc.tensor.matmul(
            s_ps[:, 512:1024],
            lhsT=kt_b[64:128, kt_i * P : (kt_i + 1) * P],
            rhs=qt_b[64:128, q0 + 512 : q0 + 1024],
            start=True,
            stop=True,
            tile_position=(64, 0),
        )
        dst = p_b[:, kt_i, q0 : q0 + 1024]
        if kt_i in ACT_KT:
            nc.scalar.activation(
                dst, s_ps[:], mybir.ActivationFunctionType.Exp, scale=float(SCALE)
            )
        else:
            nc.vector.tensor_scalar(
                dst.bitcast(i16),
                s_ps[:],
                SCH_A,
                SCH_B,
                mybir.AluOpType.mult,
                mybir.AluOpType.add,
            )


def emit_pv(nc, pools, aps, h, p_b, v_b, qcs):
    """PV + epilogue for the given q-chunks of head h."""
    qt, kt, vp, out = aps
    qk_pool, v_pool, p_pool, epi_pool, ps_s, ps_o, ps_tr, ident = pools
    for qc in qcs:
        o_ps = ps_o.tile([P, 512], f32, tag="o")
        for kt_i in range(KT):
            nc.tensor.matmul(
                o_ps[: D + 1, :],
                lhsT=v_b[:, kt_i, :],
                rhs=p_b[:, kt_i, qc * 512 : (qc + 1) * 512],
                start=(kt_i == 0),
                stop=(kt_i == KT - 1),
            )
        ot_sb = epi_pool.tile([D + 1, 512], f32, tag="ot")
        nc.vector.tensor_copy(ot_sb[:], o_ps[: D + 1, :])
        for c in range(4):
            tr_ps = ps_tr.tile([P, D + 1], f32, tag="tr")
            nc.tensor.transpose(
                tr_ps[:], ot_sb[:, c * P : (c + 1) * P], ident[: D + 1, : D + 1]
            )
            rden = epi_pool.tile([P, 1], f32, tag="rden")
            nc.vector.reciprocal_approx_fast(rden[:], tr_ps[:, D : D + 1])
            o_sb = epi_pool.tile([P, D], f32, tag="osb")
            nc.vector.tensor_mul(
                o_sb[:], tr_ps[:, :D], rden[:, 0:1].to_broadcast((P, D))
            )
            q0 = qc * 512 + c * P
            nc.sync.dma_start(out[h, q0 : q0 + P, :], o_sb[:])


def build_nc():
    nc = bacc.Bacc("TRN2", target_bir_lowering=False, debug=False)
    qt = nc.dram_tensor("qt", [HPC, P, S], bf16, kind="ExternalInput").ap()
    kt = nc.dram_tensor("kt", [HPC, P, S], bf16, kind="ExternalInput").ap()
    vp = nc.dram_tensor("vp", [HPC, P, KT, D + 1], bf16, kind="ExternalInput").ap()
    out = nc.dram_tensor("out", [HPC, S, D], f32, kind="ExternalOutput").ap()
    aps = (qt, kt, vp, out)

    with tile.TileContext(nc) as tc:
        with (
            tc.tile_pool(name="const", bufs=1) as const_pool,
            tc.tile_pool(name="qk", bufs=2) as qk_pool,
            tc.tile_pool(name="v", bufs=2) as v_pool,
            tc.tile_pool(name="p", bufs=2) as p_pool,
            tc.tile_pool(name="epi", bufs=3) as epi_pool,
            tc.tile_pool(name="ps_s", bufs=3, space="PSUM") as ps_s,
            tc.tile_pool(name="ps_o", bufs=1, space="PSUM") as ps_o,
            tc.tile_pool(name="ps_tr", bufs=1, space="PSUM") as ps_tr,
        ):
            ident = const_pool.tile([P, P], f32)
            make_identity(nc, ident[:])
            pools = (qk_pool, v_pool, p_pool, epi_pool, ps_s, ps_o, ps_tr, ident)

            # Pipeline: head h's scores interleave with head h-1's PV so both
            # exp engines (ScalarE+VectorE) and the PE stay saturated.
            prev = None
            for h in range(HPC):
                qt_b, kt_b, v_b, p_b = emit_loads(nc, pools, aps, h)
                emit_scores_half(nc, pools, h, 0, qt_b, kt_b, p_b)
                if prev is not None:
                    emit_pv(nc, pools, aps, h - 1, prev[0], prev[1], [0, 1])
                emit_scores_half(nc, pools, h, 1, qt_b, kt_b, p_b)
                if prev is not None:
                    emit_pv(nc, pools, aps, h - 1, prev[0], prev[1], [2, 3])
                prev = (p_b, v_b)
            emit_pv(nc, pools, aps, HPC - 1, prev[0], prev[1], [0, 1])
            emit_pv(nc, pools, aps, HPC - 1, prev[0], prev[1], [2, 3])

    nc.compile()
    return nc


def shard_inputs(Q, K, V):
    """Full [B,H,S,D] f32 -> per-core input maps (layout + dtype choices)."""
    Qh = np.asarray(Q, dtype=np.float32).reshape(B * H, S, D)
    Kh = np.asarray(K, dtype=np.float32).reshape(B * H, S, D)
    Vh = np.asarray(V, dtype=np.float32).reshape(B * H, S, D)

    in_maps = []
    for c in range(NCORES):
        sl = slice(c * HPC, (c + 1) * HPC)
        qt = np.empty((HPC, P, S), dtype=ml_dtypes.bfloat16)
        kt = np.empty((HPC, P, S), dtype=ml_dtypes.bfloat16)
        qt[:, :D, :] = Qh[sl].transpose(0, 2, 1).astype(ml_dtypes.bfloat16)
        kt[:, :D, :] = Kh[sl].transpose(0, 2, 1).astype(ml_dtypes.bfloat16)
        qt[:, D:, :] = qt[:, :D, :]  # duplicate for row-group 64-127
        kt[:, D:, :] = kt[:, :D, :]
        vp = np.ones((HPC, S, D + 1), dtype=np.float32)
        vp[:, :, :D] = Vh[sl]
        # [h, (kt p), d] -> [h, p, kt, d']
        vp = (
            vp.reshape(HPC, KT, P, D + 1)
            .transpose(0, 2, 1, 3)
            .astype(ml_dtypes.bfloat16)
        )
        in_maps.append({"qt": np.ascontiguousarray(qt),
                        "kt": np.ascontiguousarray(kt),
                        "vp": np.ascontiguousarray(vp)})
    return in_maps


_NC_CACHE = None


def kernel(Q, K, V):
    global _NC_CACHE
    if _NC_CACHE is None:
        _NC_CACHE = build_nc()
    nc = _NC_CACHE
    in_maps = shard_inputs(Q, K, V)
    res = run_bass_kernel_spmd(nc, in_maps, core_ids=list(range(NCORES)))
    out = np.empty((B * H, S, D), dtype=np.float32)
    for c in range(NCORES):
        out[c * HPC : (c + 1) * HPC] = res.results[c]["out"]
    return out.reshape(B, H, S, D)


if __name__ == "__main__":
    nc = build_nc()
    print("compiled OK")


# revision 5
# speedup vs baseline: 1.1107x; 1.0068x over previous
"""Trainium2 8-core attention kernel (v3).

Problem: B=2, H=16, S=2048, D=64 dense attention, f32 I/O.
Sharding: B*H = 32 head-batches -> 4 heads per NeuronCore (embarrassingly
parallel, no collectives).

Per-core algorithm (transposed score space end-to-end):
  S^T[k, q] = K_dmaj . Q_dmaj      PE matmul, contraction d=64, ROW-TILED
                                   (two concurrent matmuls, row groups 0/64)
  P = exp(S^T / 8)                 hybrid exp: ScalarE ACT for 19/32 tiles,
                                   VectorE Schraudolph (f32->int16
                                   round(A*s+B) bitcast to bf16) for 13/32
  outT[d', q] = V'^T @ P           PE matmul, contraction k; V'=[V|ones] so
                                   row 64 = softmax denominator
  outT[:64] /= den                 DVE recip_approx + DMA partition-broadcast
                                   + DVE multiply; output stays [d, q] and the
                                   host transposes back (pure layout)

Host side only reshapes/transposes/casts (layout choices for sharding):
  qt, kt: [4, 128, 2048] bf16 (d on partitions, rows 64:128 duplicate 0:64)
  vp:     [4, 128, 16, 65] bf16 (k%128 on partitions, ones column appended)
  ot:     [4, 64, 2048] f32 (transposed; host transposes to [4, 2048, 64])
"""

import numpy as np
import ml_dtypes

import concourse.bass as bass
import concourse.tile as tile
from concourse import bacc, mybir
from concourse.bass_utils import run_bass_kernel_spmd

B, H, S, D = 2, 16, 2048, 64
NCORES = 8
HPC = (B * H) // NCORES  # heads per core = 4
P = 128
KT = S // P  # 16 k-tiles
SCALE = 1.0 / np.sqrt(D)  # 0.125

# Schraudolph bf16-exp constants: bits16 = round(A*s + B); bitcast -> bf16
SCH_A = float(P * np.log2(np.e) * SCALE)
SCH_B = float(P * 127 - 7.5)


def is_dve_tile(kt_i, half):
    """19 tiles on ScalarE, 13 on VectorE (error grows with DVE share)."""
    return kt_i >= 10 if half == 0 else kt_i >= 9


f32 = mybir.dt.float32
bf16 = mybir.dt.bfloat16
i16 = mybir.dt.int16


def emit_loads(nc, pools, aps, h):
    qt, kt, vp, ot = aps
    qk_pool, v_pool, p_pool, epi_pool, ps_s, ps_o = pools
    qt_b = qk_pool.tile([P, S], bf16, tag="qt")
    kt_b = qk_pool.tile([P, S], bf16, tag="kt")
    # split loads so the first QK tile's deps land early
    nc.sync.dma_start(kt_b[:, : S // 2], kt[h, :, : S // 2])
    nc.sync.dma_start(qt_b[:, : S // 2], qt[h, :, : S // 2])
    nc.sync.dma_start(kt_b[:, S // 2 :], kt[h, :, S // 2 :])
    nc.sync.dma_start(qt_b[:, S // 2 :], qt[h, :, S // 2 :])
    v_b = v_pool.tile([P, KT, D + 1], bf16, tag="v")
    nc.sync.dma_start(v_b[:], vp[h])
    p_b = p_pool.tile([P, KT, S], bf16, tag="p")
    return qt_b, kt_b, v_b, p_b


def emit_qk_tile(nc, pools, half, kt_i, qt_b, kt_b, p_b):
    """One [128, 1024] score tile: row-tiled QK pair + exp."""
    qk_pool, v_pool, p_pool, epi_pool, ps_s, ps_o = pools
    q0 = half * 1024
    s_ps = ps_s.tile([P, 1024], f32, tag="s")
    nc.tensor.matmul(
        s_ps[:, 0:512],
        lhsT=kt_b[0:64, kt_i * P : (kt_i + 1) * P],
        rhs=qt_b[0:64, q0 : q0 + 512],
        start=True,
        stop=True,
        tile_position=(0, 0),
    )
    nc.tensor.matmul(
        s_ps[:, 512:1024],
        lhsT=kt_b[64:128, kt_i * P : (kt_i + 1) * P],
        rhs=qt_b[64:128, q0 + 512 : q0 + 1024],
        start=True,
        stop=True,
        tile_position=(64, 0),
    )
    dst = p_b[:, kt_i, q0 : q0 + 1024]
    if is_dve_tile(kt_i, half):
        nc.vector.tensor_scalar(
            dst.bitcast(i16),
            s_ps[:],
            SCH_A,
            SCH_B,
            mybir.AluOpType.mult,
            mybir.AluOpType.add,
        )
    else:
        nc.scalar.activation(
            dst, s_ps[:], mybir.ActivationFunctionType.Exp, scale=float(SCALE)
        )


def emit_pv_qc(nc, pools, aps, h, p_b, v_b, qc):
    """PV accumulation + division epilogue for one 512-wide q-chunk."""
    qt, kt, vp, ot = aps
    qk_pool, v_pool, p_pool, epi_pool, ps_s, ps_o = pools
    o_ps = ps_o.tile([P, 512], f32, tag="o")
    for kt_i in range(KT):
        nc.tensor.matmul(
            o_ps[: D + 1, :],
            lhsT=v_b[:, kt_i, :],
            rhs=p_b[:, kt_i, qc * 512 : (qc + 1) * 512],
            start=(kt_i == 0),
            stop=(kt_i == KT - 1),
        )
    den = epi_pool.tile([1, 512], f32, tag="den")
    nc.vector.tensor_copy(den[:], o_ps[D : D + 1, :])
    rden = epi_pool.tile([1, 512], f32, tag="rden")
    nc.vector.reciprocal_approx_fast(rden[:], den[:])
    rden_bc = epi_pool.tile([D, 512], f32, tag="rbc")
    nc.gpsimd.partition_broadcast(rden_bc[:], rden[0:1, :])
    ot_sb = epi_pool.tile([D, 512], f32, tag="ot")
    nc.vector.tensor_mul(ot_sb[:], o_ps[:D, :], rden_bc[:])
    nc.sync.dma_start(ot[h, :, qc * 512 : (qc + 1) * 512], ot_sb[:])


def build_nc():
    nc = bacc.Bacc("TRN2", target_bir_lowering=False, debug=False)
    qt = nc.dram_tensor("qt", [HPC, P, S], bf16, kind="ExternalInput").ap()
    kt = nc.dram_tensor("kt", [HPC, P, S], bf16, kind="ExternalInput").ap()
    vp = nc.dram_tensor("vp", [HPC, P, KT, D + 1], bf16, kind="ExternalInput").ap()
    ot = nc.dram_tensor("ot", [HPC, D, S], f32, kind="ExternalOutput").ap()
    aps = (qt, kt, vp, ot)

    with tile.TileContext(nc) as tc:
        with (
            tc.tile_pool(name="qk", bufs=2) as qk_pool,
            tc.tile_pool(name="v", bufs=2) as v_pool,
            tc.tile_pool(name="p", bufs=2) as p_pool,
            tc.tile_pool(name="epi", bufs=3) as epi_pool,
            tc.tile_pool(name="ps_s", bufs=3, space="PSUM") as ps_s,
            tc.tile_pool(name="ps_o", bufs=2, space="PSUM") as ps_o,
        ):
            pools = (qk_pool, v_pool, p_pool, epi_pool, ps_s, ps_o)

            # Software pipeline: head h's QK/exp stream is interleaved (at kt
            # granularity) with head h-1's PV chunks so the PE fills its
            # exp-throttled stall slots with PV matmuls.
            prev = None
            for h in range(HPC):
                qt_b, kt_b, v_b, p_b = emit_loads(nc, pools, aps, h)
                for half in range(2):
                    for kt_i in range(KT):
                        emit_qk_tile(nc, pools, half, kt_i, qt_b, kt_b, p_b)
                        if prev is not None:
                            if kt_i == 5:
                                emit_pv_qc(
                                    nc, pools, aps, h - 1, *prev, 2 * half
                                )
                            elif kt_i == 11:
                                emit_pv_qc(
                                    nc, pools, aps, h - 1, *prev, 2 * half + 1
                                )
                prev = (p_b, v_b)
            for qc in range(4):
                emit_pv_qc(nc, pools, aps, HPC - 1, *prev, qc)

    nc.compile()
    return nc


def shard_inputs(Q, K, V):
    """Full [B,H,S,D] f32 -> per-core input maps (layout + dtype choices)."""
    Qh = np.asarray(Q, dtype=np.float32).reshape(B * H, S, D)
    Kh = np.asarray(K, dtype=np.float32).reshape(B * H, S, D)
    Vh = np.asarray(V, dtype=np.float32).reshape(B * H, S, D)

    in_maps = []
    for c in range(NCORES):
        sl = slice(c * HPC, (c + 1) * HPC)
        qt = np.empty((HPC, P, S), dtype=ml_dtypes.bfloat16)
        kt = np.empty((HPC, P, S), dtype=ml_dtypes.bfloat16)
        qt[:, :D, :] = Qh[sl].transpose(0, 2, 1).astype(ml_dtypes.bfloat16)
        kt[:, :D, :] = Kh[sl].transpose(0, 2, 1).astype(ml_dtypes.bfloat16)
        qt[:, D:, :] = qt[:, :D, :]  # duplicate for row-group 64-127
        kt[:, D:, :] = kt[:, :D, :]
        vp = np.ones((HPC, S, D + 1), dtype=np.float32)
        vp[:, :, :D] = Vh[sl]
        # [h, (kt p), d] -> [h, p, kt, d']
        vp = (
            vp.reshape(HPC, KT, P, D + 1)
            .transpose(0, 2, 1, 3)
            .astype(ml_dtypes.bfloat16)
        )
        in_maps.append({"qt": np.ascontiguousarray(qt),
                        "kt": np.ascontiguousarray(kt),
                        "vp": np.ascontiguousarray(vp)})
    return in_maps


_NC_CACHE = None


def kernel(Q, K, V):
    global _NC_CACHE
    if _NC_CACHE is None:
        _NC_CACHE = build_nc()
    nc = _NC_CACHE
    in_maps = shard_inputs(Q, K, V)
    res = run_bass_kernel_spmd(nc, in_maps, core_ids=list(range(NCORES)))
    out = np.empty((B * H, S, D), dtype=np.float32)
    for c in range(NCORES):
        out[c * HPC : (c + 1) * HPC] = res.results[c]["ot"].transpose(0, 2, 1)
    return out.reshape(B, H, S, D)


if __name__ == "__main__":
    nc = build_nc()
    print("compiled OK")
